# revision 47
# baseline (speedup 1.0000x reference)
"""DeepSet-equivariant layer on 8 TRN2 NeuronCores.

Math (reference):
    y = x @ w1 + (colsum(x) @ w2) / n + bias        x: (n, 128)

Distribution: shard x and y along the set dimension n across the 8 cores;
w1/w2/bias replicated. Each core exchanges its local 128-long colsum with
ONE remote_dma_broadcast (SBUF -> 7 peers' SBUF, a few us, bypassing the
~90us ncfw collective path).

v8 schedule (per core):
  phase 1 (input window, ~18.5us at ~400 GB/s):
    - xT streams bf16 into resident SBUF, chunks alternating over BOTH
      HWDGE rings (each ring alone sustains only ~200 GB/s).
    - colsum computed DIRECTLY from x, split across DVE (reduce_sum) and
      ACT (copy->trash with accum_out), ~13us each — no PE dependency, so
      the colsum completes right at input-end and the exchange fires.
  exchange: local colsum -> gather slot 0 -> remote_dma_broadcast to
    slot 8+my_id on 7 peers; wait 14 remote-sem incs; t = Sx @ (w2/n) + b.
    PE-warmup dummies are gated on a LATE input chunk so they run inside
    the exchange wait and the burst below starts at 2.4 GHz, not 1.2.
  phase 2 (dense burst, drain/DMA-bound ~19us): per 2048-col block,
    4 matmuls (512 each) into two independent PSUM tensors (psA/psB);
    DVE drains psA with +t fused (tensor_scalar scalar1=t), ACT drains
    psB with +t (activation Identity bias=t) — both 1x from PSUM, running
    concurrently on disjoint lanes; per-block output DMA alternates rings.
    The dense matmul stream keeps the PE HAM clock-gate open (input-paced
    matmuls flicker between 1.2/2.4 GHz and become the pole), and blocks
    0/1-psA and 0-psB are prefilled during the exchange wait.

Measured (8-core traces): input 18-20us at 360-430 GB/s on fast cores,
colsum trails input by <1us, exchange wait absorbs launch skew, burst
~15us, post-t critical path = out-DMA (~16.5us) + ~4.5us Tile epilogue.
Per-core exec 62-75us; the reported number is whichever profiled core
waited longest for the slowest peer's colsum (launch skew, up to ~15us).

Tile's single-core scheduling sim cannot model remote arrivals, so the
two protocol waits are emitted after the TileContext and spliced into
engine-queue position by direct BIR list surgery.
"""

import numpy as np
import ml_dtypes

import concourse.bass as bass
import concourse.tile as tile
from concourse import bacc, mybir
from concourse.bass_utils import run_bass_kernel_spmd

N_CORES = 8
D = 128                 # d_in == d_out
N_ROWS = 200000         # full set size
R = 25088               # padded rows per core: 8 * 25088 = 200704 >= 200000
PS_N = 2048             # columns per compute block
MM_N = 512              # moving-operand free dim per matmul (1 PSUM bank)
HALF = 1024             # block half: psA (DVE lane) | psB (ACT lane)
WARM_N = 512            # dummy-matmul width

F32 = mybir.dt.float32
BF16 = mybir.dt.bfloat16
NP_BF16 = ml_dtypes.bfloat16

GATHER_SLOTS = 16       # slot 0: local colsum(x); slots 8+sender: remote

# input DMA chunks: small first chunk (needed by nothing — just starts the
# rings), big middle, tapered tail so the last colsum piece is short.
IN_WIDTHS = [2048, 2048, 4096, 4096, 4096, 4096, 2048, 1024, 1024, 512]
assert sum(IN_WIDTHS) == R

# colsum sub-chunks. reduce_sum on DVE/GpSimd reads x and writes only a
# column; the ACT path (copy+accum_out) also writes a trash tile, and its
# SBUF write traffic contends with the input-DMA writes (v8 measured the
# input rate sagging 417->230 GB/s once the copies started). So: DVE does
# most, idle GpSimd takes three early sub-chunks, ACT three late ones
# with an fp8 trash target to halve its writes.
CS_WIDTHS = [2048] * 12 + [512]
assert sum(CS_WIDTHS) == R
CS_ACT = {3, 5, 7, 9, 11}   # (GpSimd tensor_reduce is partition-axis only)

# compute blocks (phase 2): 12 x 2048 + 512 tail (tail = psA lane only)
TILE_WIDTHS = [PS_N] * 12 + [512]
assert sum(TILE_WIDTHS) == R

N_WARM_MM = 5           # ~3.2us of gated dummies bridge the exchange wait
WARM_GATE = 22528       # dummies read x here (chunk 7) -> start ~2 chunks
                        # before input-end, PE warm when the burst begins


def _offsets(widths):
    out, c0 = [], 0
    for w in widths:
        out.append((c0, w))
        c0 += w
    return out


def _move_before(nc, inst, target):
    """Move a post-TileContext instruction directly before `target` in the
    block that holds it (engine dispatch follows list order per engine)."""
    src = dst = None
    for bb in nc.m.functions[0].blocks:
        names = [i.name for i in bb.instructions]
        if inst.name in names:
            src = bb
        if target.name in names:
            dst = bb
    assert src is not None and dst is not None
    src.instructions.remove(inst)
    dst.instructions.insert(dst.instructions.index(target), inst)


def _move_after(nc, inst, target):
    src = dst = None
    for bb in nc.m.functions[0].blocks:
        names = [i.name for i in bb.instructions]
        if inst.name in names:
            src = bb
        if target.name in names:
            dst = bb
    assert src is not None and dst is not None
    src.instructions.remove(inst)
    dst.instructions.insert(dst.instructions.index(target) + 1, inst)


def build_nc(r: int):
    in_chunks = _offsets(IN_WIDTHS)
    cs_chunks = _offsets(CS_WIDTHS)
    blocks = _offsets(TILE_WIDTHS)

    nc = bacc.Bacc(
        "TRN2",
        target_bir_lowering=False,
        debug=False,
        num_devices=N_CORES,
    )

    xt = nc.declare_dram_parameter("xt", [D, r], BF16, isOutput=False)
    # wpack (bf16): cols 0..127 = w1, 128..255 = w2/n, col 256 = bias,
    # col 257 = pad. One 516B-per-partition DMA.
    wpack = nc.declare_dram_parameter("wpack", [D, 2 * D + 2], BF16, isOutput=False)
    out = nc.declare_dram_parameter("out", [D, r], BF16, isOutput=True)

    # Dummy collective for rank-coordinated launch; nothing waits on it.
    ccw_in = nc.dram_tensor("ccw_in", [D, 1], F32)
    ccw_out = nc.dram_tensor("ccw_out", [D, 1], F32, addr_space="Shared")
    warm_sem = nc.alloc_semaphore("warm_cc")
    nc.gpsimd.collective_compute(
        "AllReduce",
        mybir.AluOpType.add,
        replica_groups=[list(range(N_CORES))],
        ins=[ccw_in.ap().opt()],
        outs=[ccw_out.ap().opt()],
    ).then_inc(warm_sem)

    gsem = nc.alloc_semaphore("gather_sem")
    lsem = nc.alloc_semaphore("rdma_local")
    cs_sem = nc.alloc_semaphore("cs_done")

    # Fixed-address gather buffer (remote cores write slots 8..15).
    gather_sb = nc.alloc_sbuf_tensor("gather_sb", [D, GATHER_SLOTS], F32)

    n_cs = len(cs_chunks)

    with tile.TileContext(nc) as tc:
        with (
            tc.tile_pool(name="const", bufs=1) as const_pool,
            tc.tile_pool(name="xres", bufs=1) as xres_pool,
            tc.tile_pool(name="obuf", bufs=1) as obuf_pool,
            tc.tile_pool(name="mma", bufs=2, space=bass.MemorySpace.PSUM) as mma_pool,
            tc.tile_pool(name="mmb", bufs=2, space=bass.MemorySpace.PSUM) as mmb_pool,
        ):
            wpack_sb = const_pool.tile([D, 2 * D + 2], BF16)
            w1_sb = wpack_sb[:, 0:D]
            w2n_sb = wpack_sb[:, D : 2 * D]
            bias_bf = wpack_sb[:, 2 * D : 2 * D + 1]
            bias_sb = const_pool.tile([D, 1], F32)
            cs_parts = const_pool.tile([D, n_cs], F32)
            trash = const_pool.tile([D, 2048], mybir.dt.float8e4)
            t_sb = const_pool.tile([D, 1], F32)

            nc.gpsimd.memset(gather_sb[:, :], 0.0)

            # broadcast destination slot offset: 8 + my core id (elements)
            off_gp = nc.gpsimd.alloc_register("slot_off")
            nc.gpsimd.reg_load(off_gp, nc.partition_id_tensor[0:1, 0:1])
            nc.gpsimd.reg_add(off_gp, off_gp, 8)

            # one broadcast: my slot 0 -> peers' slot 8+my_id (self = None)
            slot_out = bass.AP(gather_sb, off_gp, [[GATHER_SLOTS, D], [1, 1]])
            rdests = [None] + [(0, k) for k in range(1, N_CORES)]
            nc.gpsimd.remote_dma_broadcast(
                slot_out,
                gather_sb[:, 0:1],
                gsem,
                lsem,
                rdests=rdests,
            )

            # weights first on Sync, then the input stream on both rings
            nc.sync.dma_start(wpack_sb[:], wpack[:, :])
            # bias needs fp32 in SBUF (tensor_scalar scalars are f32)
            nc.scalar.activation(
                bias_sb[:], bias_bf, mybir.ActivationFunctionType.Copy
            )

            x_sb = xres_pool.tile([D, r], BF16)
            for c, (c0, cw) in enumerate(in_chunks):
                eng = nc.sync if c % 2 == 0 else nc.scalar
                eng.dma_start(x_sb[:, c0 : c0 + cw], xt[:, c0 : c0 + cw])

            # phase 1 colsum from x, split DVE / GpSimd / ACT
            for j, (c0, cw) in enumerate(cs_chunks):
                if j in CS_ACT:
                    nc.scalar.activation(
                        trash[:, :cw],
                        x_sb[:, c0 : c0 + cw],
                        mybir.ActivationFunctionType.Copy,
                        accum_out=cs_parts[:, j : j + 1],
                    )
                else:
                    nc.vector.reduce_sum(
                        cs_parts[:, j : j + 1],
                        x_sb[:, c0 : c0 + cw],
                        axis=mybir.AxisListType.X,
                    )

            # local colsum -> gather slot 0, gate + fire the exchange
            cs_reduce = nc.vector.reduce_sum(
                gather_sb[:, 0:1], cs_parts[:], axis=mybir.AxisListType.X
            )
            trig = nc.gpsimd.trigger_dma(
                count=None, signals_writable=[gather_sb[:, :]]
            )

            # PE warmup dummies (write psB slots, released at once). Gated
            # on BOTH a late input chunk (data) and — via a spliced wait
            # below — the FIRST peer arrival (gsem>=2): on early-launch
            # cores that lands mid-wait, so the PE is warm when t arrives;
            # on late cores it is already satisfied and adds no delay.
            first_dummy = None
            for k in range(N_WARM_MM):
                warm_ps = mmb_pool.tile([D, WARM_N], F32, tag="psB")
                g0 = WARM_GATE + (k % 2) * WARM_N
                mm = nc.tensor.matmul(
                    warm_ps[:, :], x_sb[:, g0 : g0 + D], x_sb[:, g0 : g0 + WARM_N]
                )
                if first_dummy is None:
                    first_dummy = mm

            # prefill psA blocks 0/1 + psB block 0 during the wait (no t
            # dependency; t_ps below takes the second psB slot, so block
            # 1's psB matmuls correctly queue behind the block-0 drain)
            pre_a = []
            for i in (0, 1):
                c0, cw = blocks[i]
                ps_a = mma_pool.tile([D, HALF], F32, tag="psA")
                for s0 in range(0, HALF, MM_N):
                    nc.tensor.matmul(
                        ps_a[:, s0 : s0 + MM_N],
                        w1_sb[:],
                        x_sb[:, c0 + s0 : c0 + s0 + MM_N],
                    )
                pre_a.append(ps_a)
            pre_b0 = mmb_pool.tile([D, HALF], F32, tag="psB")
            for s0 in range(HALF, PS_N, MM_N):
                nc.tensor.matmul(
                    pre_b0[:, s0 - HALF : s0 - HALF + MM_N],
                    w1_sb[:],
                    x_sb[:, blocks[0][0] + s0 : blocks[0][0] + s0 + MM_N],
                )

            # global colsum -> t = Sx @ (w2/n) + bias (bf16 matmul)
            sxg = const_pool.tile([D, 1], BF16)
            with nc.allow_low_precision(
                reason="Sx quantized to bf16 for the t-matmul; the transmit "
                "term is ~0.2% of y and bf16 adds ~0.4% relative to it"
            ):
                gcs_reduce = nc.vector.reduce_sum(
                    sxg[:], gather_sb[:, :], axis=mybir.AxisListType.X
                )
            t_ps = mmb_pool.tile([D, WARM_N], F32, tag="psB")
            nc.tensor.matmul(t_ps[:, :1], w2n_sb, sxg[:])
            nc.vector.tensor_scalar(
                out=t_sb[:],
                in0=t_ps[:, :1],
                scalar1=bias_sb[:],
                scalar2=None,
                op0=mybir.AluOpType.add,
            )

            # phase 2: dense burst — mm, drain+t on two PSUM lanes, DMA out
            ob = obuf_pool.tile([D, r], BF16)
            for i, (c0, cw) in enumerate(blocks):
                h = min(HALF, cw)
                if i < 2:
                    ps_a = pre_a[i]
                else:
                    ps_a = mma_pool.tile([D, HALF], F32, tag="psA")
                    for s0 in range(0, h, MM_N):
                        sw = min(MM_N, h - s0)
                        nc.tensor.matmul(
                            ps_a[:, s0 : s0 + sw],
                            w1_sb[:],
                            x_sb[:, c0 + s0 : c0 + s0 + sw],
                        )
                if cw > h:
                    if i == 0:
                        ps_b = pre_b0
                    else:
                        ps_b = mmb_pool.tile([D, HALF], F32, tag="psB")
                        for s0 in range(h, cw, MM_N):
                            nc.tensor.matmul(
                                ps_b[:, s0 - h : s0 - h + MM_N],
                                w1_sb[:],
                                x_sb[:, c0 + s0 : c0 + s0 + MM_N],
                            )
                # DVE lane: ob = psA + t (1x from PSUM)
                nc.vector.tensor_scalar(
                    out=ob[:, c0 : c0 + h],
                    in0=ps_a[:, :h],
                    scalar1=t_sb[:],
                    scalar2=None,
                    op0=mybir.AluOpType.add,
                )
                if cw > h:
                    # ACT lane: ob = psB + t
                    nc.scalar.activation(
                        ob[:, c0 + h : c0 + cw],
                        ps_b[:, : cw - h],
                        mybir.ActivationFunctionType.Identity,
                        bias=t_sb[:],
                        scale=1.0,
                    )
                (nc.sync if i % 2 == 0 else nc.scalar).dma_start(
                    out[:, c0 : c0 + cw], ob[:, c0 : c0 + cw]
                )

    # Protocol signal + waits, invisible to Tile's scheduling sim:
    #  - cs_sem inc right after the colsum reduce on DVE
    #  - trigger must not fire before the local colsum is written
    #  - the gather reduce must not read before all 7 remote slots landed
    inc_cs = nc.vector.sem_inc(cs_sem, 1)
    _move_after(nc, inc_cs.ins, cs_reduce.ins)
    w_cs = nc.gpsimd.wait_ge(cs_sem, 1)
    _move_before(nc, w_cs.ins, trig.ins)
    w_arr = nc.vector.wait_ge(gsem, 14)
    _move_before(nc, w_arr.ins, gcs_reduce.ins)
    w_warm = nc.tensor.wait_ge(gsem, 2)
    _move_before(nc, w_warm.ins, first_dummy.ins)

    nc.compile()
    return nc


_nc_cache: dict = {}


def _get_nc(r: int):
    if r not in _nc_cache:
        _nc_cache[r] = build_nc(r)
    return _nc_cache[r]


LAST_RESULTS = None


def _execute(x, w1, w2, bias, r, trace=False, tmpdir=None, trace_cores=None):
    global LAST_RESULTS
    x = np.ascontiguousarray(np.asarray(x, dtype=np.float32))
    w1 = np.ascontiguousarray(np.asarray(w1, dtype=np.float32))
    w2 = np.ascontiguousarray(np.asarray(w2, dtype=np.float32))
    bias = np.asarray(bias, dtype=np.float32)
    n, d = x.shape
    assert d == D and r * N_CORES >= n

    xp = np.zeros((N_CORES * r, d), dtype=np.float32)
    xp[:n] = x
    # (8, r, d) -> (8, d, r) pre-transposed bf16 shards
    xts = np.ascontiguousarray(
        xp.reshape(N_CORES, r, d).transpose(0, 2, 1)
    ).astype(NP_BF16)
    wpack = np.ascontiguousarray(
        np.concatenate(
            [
                w1,
                w2 / float(n),
                bias.reshape(1, d).T,
                np.zeros((d, 1), np.float32),
            ],
            axis=1,
        )
    ).astype(NP_BF16)

    in_maps = [{"xt": xts[i], "wpack": wpack} for i in range(N_CORES)]

    nc = _get_nc(r)
    kwargs = {}
    if trace:
        kwargs.update(trace=True, tmpdir=tmpdir)
        if trace_cores is not None:
            kwargs.update(trace_cores=trace_cores)

    # A wedged device / flaky launch occasionally corrupts one core's run
    # (non-finite or wildly out-of-range outputs) or raises outright.
    # Detect and re-execute: y values are O(100), so 1e4 is a generous
    # sanity bound.
    y = None
    for attempt in range(3):
        try:
            res = run_bass_kernel_spmd(
                nc, in_maps, core_ids=list(range(N_CORES)), **kwargs
            )
        except Exception:
            if attempt == 2:
                raise
            continue
        LAST_RESULTS = res
        yts = [res.results[i]["out"] for i in range(N_CORES)]  # (D, r) bf16
        y = np.concatenate([yt.T for yt in yts], axis=0)[:n].astype(np.float32)
        if np.isfinite(y).all() and np.abs(y).max() < 1e4:
            break
    assert y is not None
    return np.ascontiguousarray(y)


def kernel(x, w1, w2, bias):
    return _execute(x, w1, w2, bias, R)


# revision 48
# speedup vs baseline: 1.0331x; 1.0331x over previous
"""DeepSet-equivariant layer on 8 TRN2 NeuronCores.

Math (reference):
    y = x @ w1 + (colsum(x) @ w2) / n + bias        x: (n, 128)

Distribution: shard x and y along the set dimension n across the 8 cores;
w1/w2/bias replicated. Each core exchanges its local 128-long colsum with
ONE remote_dma_broadcast (SBUF -> 7 peers' SBUF, a few us, bypassing the
~90us ncfw collective path).

v8 schedule (per core):
  phase 1 (input window, ~18.5us at ~400 GB/s):
    - xT streams bf16 into resident SBUF, chunks alternating over BOTH
      HWDGE rings (each ring alone sustains only ~200 GB/s).
    - colsum computed DIRECTLY from x, split across DVE (reduce_sum) and
      ACT (copy->trash with accum_out), ~13us each — no PE dependency, so
      the colsum completes right at input-end and the exchange fires.
  exchange: local colsum -> gather slot 0 -> remote_dma_broadcast to
    slot 8+my_id on 7 peers; wait 14 remote-sem incs; t = Sx @ (w2/n) + b.
    PE-warmup dummies are gated on a LATE input chunk so they run inside
    the exchange wait and the burst below starts at 2.4 GHz, not 1.2.
  phase 2 (dense burst, drain/DMA-bound ~19us): per 2048-col block,
    4 matmuls (512 each) into two independent PSUM tensors (psA/psB);
    DVE drains psA with +t fused (tensor_scalar scalar1=t), ACT drains
    psB with +t (activation Identity bias=t) — both 1x from PSUM, running
    concurrently on disjoint lanes; per-block output DMA alternates rings.
    The dense matmul stream keeps the PE HAM clock-gate open (input-paced
    matmuls flicker between 1.2/2.4 GHz and become the pole), and blocks
    0/1-psA and 0-psB are prefilled during the exchange wait.

Measured (8-core traces): input 18-20us at 360-430 GB/s on fast cores,
colsum trails input by <1us, exchange wait absorbs launch skew, burst
~15us, post-t critical path = out-DMA (~16.5us) + ~4.5us Tile epilogue.
Per-core exec 62-75us; the reported number is whichever profiled core
waited longest for the slowest peer's colsum (launch skew, up to ~15us).

Tile's single-core scheduling sim cannot model remote arrivals, so the
two protocol waits are emitted after the TileContext and spliced into
engine-queue position by direct BIR list surgery.
"""

import numpy as np
import ml_dtypes

import concourse.bass as bass
import concourse.tile as tile
from concourse import bacc, mybir
from concourse.bass_utils import run_bass_kernel_spmd

N_CORES = 8
D = 128                 # d_in == d_out
N_ROWS = 200000         # full set size
R = 25088               # padded rows per core: 8 * 25088 = 200704 >= 200000
PS_N = 2048             # columns per compute block
MM_N = 512              # moving-operand free dim per matmul (1 PSUM bank)
HALF = 1024             # block half: psA (DVE lane) | psB (ACT lane)
WARM_N = 512            # dummy-matmul width

F32 = mybir.dt.float32
BF16 = mybir.dt.bfloat16
NP_BF16 = ml_dtypes.bfloat16

GATHER_SLOTS = 16       # slot 0: local colsum(x); slots 8+sender: remote

# input DMA chunks: small first chunk (needed by nothing — just starts the
# rings), big middle, tapered tail so the last colsum piece is short.
IN_WIDTHS = [2048, 2048, 4096, 4096, 4096, 4096, 2048, 1024, 1024, 512]
assert sum(IN_WIDTHS) == R

# colsum sub-chunks. reduce_sum on DVE/GpSimd reads x and writes only a
# column; the ACT path (copy+accum_out) also writes a trash tile, and its
# SBUF write traffic contends with the input-DMA writes (v8 measured the
# input rate sagging 417->230 GB/s once the copies started). So: DVE does
# most, idle GpSimd takes three early sub-chunks, ACT three late ones
# with an fp8 trash target to halve its writes.
CS_WIDTHS = [2048] * 12 + [512]
assert sum(CS_WIDTHS) == R
CS_ACT = {3, 5, 7, 9, 11}   # (GpSimd tensor_reduce is partition-axis only)

# compute blocks (phase 2): 12 x 2048 + 512 tail (tail = psA lane only)
TILE_WIDTHS = [PS_N] * 12 + [512]
assert sum(TILE_WIDTHS) == R

N_WARM_MM = 7           # 7x634ns cold = ~4.4us > the ~3.4us HAM flip
                        # threshold (5 left the burst cold: measured)
WARM_GATE = 22528       # dummies read x here (chunk 7) -> start ~2 chunks
                        # before input-end, PE warm when the burst begins


def _offsets(widths):
    out, c0 = [], 0
    for w in widths:
        out.append((c0, w))
        c0 += w
    return out


def _move_before(nc, inst, target):
    """Move a post-TileContext instruction directly before `target` in the
    block that holds it (engine dispatch follows list order per engine)."""
    src = dst = None
    for bb in nc.m.functions[0].blocks:
        names = [i.name for i in bb.instructions]
        if inst.name in names:
            src = bb
        if target.name in names:
            dst = bb
    assert src is not None and dst is not None
    src.instructions.remove(inst)
    dst.instructions.insert(dst.instructions.index(target), inst)


def _move_after(nc, inst, target):
    src = dst = None
    for bb in nc.m.functions[0].blocks:
        names = [i.name for i in bb.instructions]
        if inst.name in names:
            src = bb
        if target.name in names:
            dst = bb
    assert src is not None and dst is not None
    src.instructions.remove(inst)
    dst.instructions.insert(dst.instructions.index(target) + 1, inst)


def build_nc(r: int):
    in_chunks = _offsets(IN_WIDTHS)
    cs_chunks = _offsets(CS_WIDTHS)
    blocks = _offsets(TILE_WIDTHS)

    nc = bacc.Bacc(
        "TRN2",
        target_bir_lowering=False,
        debug=False,
        num_devices=N_CORES,
    )

    xt = nc.declare_dram_parameter("xt", [D, r], BF16, isOutput=False)
    # wpack (bf16): cols 0..127 = w1, 128..255 = w2/n, col 256 = bias,
    # col 257 = pad. One 516B-per-partition DMA.
    wpack = nc.declare_dram_parameter("wpack", [D, 2 * D + 2], BF16, isOutput=False)
    out = nc.declare_dram_parameter("out", [D, r], BF16, isOutput=True)

    # Dummy collective for rank-coordinated launch; nothing waits on it.
    ccw_in = nc.dram_tensor("ccw_in", [D, 1], F32)
    ccw_out = nc.dram_tensor("ccw_out", [D, 1], F32, addr_space="Shared")
    warm_sem = nc.alloc_semaphore("warm_cc")
    nc.gpsimd.collective_compute(
        "AllReduce",
        mybir.AluOpType.add,
        replica_groups=[list(range(N_CORES))],
        ins=[ccw_in.ap().opt()],
        outs=[ccw_out.ap().opt()],
    ).then_inc(warm_sem)

    gsem = nc.alloc_semaphore("gather_sem")
    lsem = nc.alloc_semaphore("rdma_local")
    cs_sem = nc.alloc_semaphore("cs_done")

    # Fixed-address gather buffer (remote cores write slots 8..15).
    gather_sb = nc.alloc_sbuf_tensor("gather_sb", [D, GATHER_SLOTS], F32)

    n_cs = len(cs_chunks)

    with tile.TileContext(nc) as tc:
        with (
            tc.tile_pool(name="const", bufs=1) as const_pool,
            tc.tile_pool(name="xres", bufs=1) as xres_pool,
            tc.tile_pool(name="obuf", bufs=1) as obuf_pool,
            tc.tile_pool(name="mma", bufs=2, space=bass.MemorySpace.PSUM) as mma_pool,
            tc.tile_pool(name="mmb", bufs=2, space=bass.MemorySpace.PSUM) as mmb_pool,
        ):
            wpack_sb = const_pool.tile([D, 2 * D + 2], BF16)
            w1_sb = wpack_sb[:, 0:D]
            w2n_sb = wpack_sb[:, D : 2 * D]
            bias_bf = wpack_sb[:, 2 * D : 2 * D + 1]
            bias_sb = const_pool.tile([D, 1], F32)
            cs_parts = const_pool.tile([D, n_cs], F32)
            trash = const_pool.tile([D, 2048], mybir.dt.float8e4)
            t_sb = const_pool.tile([D, 1], F32)

            nc.gpsimd.memset(gather_sb[:, :], 0.0)

            # broadcast destination slot offset: 8 + my core id (elements)
            off_gp = nc.gpsimd.alloc_register("slot_off")
            nc.gpsimd.reg_load(off_gp, nc.partition_id_tensor[0:1, 0:1])
            nc.gpsimd.reg_add(off_gp, off_gp, 8)

            # one broadcast: my slot 0 -> peers' slot 8+my_id (self = None)
            slot_out = bass.AP(gather_sb, off_gp, [[GATHER_SLOTS, D], [1, 1]])
            rdests = [None] + [(0, k) for k in range(1, N_CORES)]
            nc.gpsimd.remote_dma_broadcast(
                slot_out,
                gather_sb[:, 0:1],
                gsem,
                lsem,
                rdests=rdests,
            )

            # weights first on Sync, then the input stream on both rings
            nc.sync.dma_start(wpack_sb[:], wpack[:, :])
            # bias needs fp32 in SBUF (tensor_scalar scalars are f32)
            nc.scalar.activation(
                bias_sb[:], bias_bf, mybir.ActivationFunctionType.Copy
            )

            x_sb = xres_pool.tile([D, r], BF16)
            for c, (c0, cw) in enumerate(in_chunks):
                eng = nc.sync if c % 2 == 0 else nc.scalar
                eng.dma_start(x_sb[:, c0 : c0 + cw], xt[:, c0 : c0 + cw])

            # phase 1 colsum from x, split DVE / GpSimd / ACT
            for j, (c0, cw) in enumerate(cs_chunks):
                if j in CS_ACT:
                    nc.scalar.activation(
                        trash[:, :cw],
                        x_sb[:, c0 : c0 + cw],
                        mybir.ActivationFunctionType.Copy,
                        accum_out=cs_parts[:, j : j + 1],
                    )
                else:
                    nc.vector.reduce_sum(
                        cs_parts[:, j : j + 1],
                        x_sb[:, c0 : c0 + cw],
                        axis=mybir.AxisListType.X,
                    )

            # local colsum -> gather slot 0, gate + fire the exchange
            cs_reduce = nc.vector.reduce_sum(
                gather_sb[:, 0:1], cs_parts[:], axis=mybir.AxisListType.X
            )
            trig = nc.gpsimd.trigger_dma(
                count=None, signals_writable=[gather_sb[:, :]]
            )

            # PE warmup dummies (write psB slots, released at once). Gated
            # on BOTH a late input chunk (data) and — via a spliced wait
            # below — the FIRST peer arrival (gsem>=2): on early-launch
            # cores that lands mid-wait, so the PE is warm when t arrives;
            # on late cores it is already satisfied and adds no delay.
            first_dummy = None
            for k in range(N_WARM_MM):
                warm_ps = mmb_pool.tile([D, WARM_N], F32, tag="psB")
                g0 = WARM_GATE + (k % 2) * WARM_N
                mm = nc.tensor.matmul(
                    warm_ps[:, :], x_sb[:, g0 : g0 + D], x_sb[:, g0 : g0 + WARM_N]
                )
                if first_dummy is None:
                    first_dummy = mm

            # prefill psA blocks 0/1 + psB block 0 during the wait (no t
            # dependency; t_ps below takes the second psB slot, so block
            # 1's psB matmuls correctly queue behind the block-0 drain)
            pre_a = []
            for i in (0, 1):
                c0, cw = blocks[i]
                ps_a = mma_pool.tile([D, HALF], F32, tag="psA")
                for s0 in range(0, HALF, MM_N):
                    nc.tensor.matmul(
                        ps_a[:, s0 : s0 + MM_N],
                        w1_sb[:],
                        x_sb[:, c0 + s0 : c0 + s0 + MM_N],
                    )
                pre_a.append(ps_a)
            pre_b0 = mmb_pool.tile([D, HALF], F32, tag="psB")
            for s0 in range(HALF, PS_N, MM_N):
                nc.tensor.matmul(
                    pre_b0[:, s0 - HALF : s0 - HALF + MM_N],
                    w1_sb[:],
                    x_sb[:, blocks[0][0] + s0 : blocks[0][0] + s0 + MM_N],
                )

            # global colsum -> t = Sx @ (w2/n) + bias (bf16 matmul)
            sxg = const_pool.tile([D, 1], BF16)
            with nc.allow_low_precision(
                reason="Sx quantized to bf16 for the t-matmul; the transmit "
                "term is ~0.2% of y and bf16 adds ~0.4% relative to it"
            ):
                gcs_reduce = nc.vector.reduce_sum(
                    sxg[:], gather_sb[:, :], axis=mybir.AxisListType.X
                )
            t_ps = mmb_pool.tile([D, WARM_N], F32, tag="psB")
            nc.tensor.matmul(t_ps[:, :1], w2n_sb, sxg[:])
            nc.vector.tensor_scalar(
                out=t_sb[:],
                in0=t_ps[:, :1],
                scalar1=bias_sb[:],
                scalar2=None,
                op0=mybir.AluOpType.add,
            )

            # phase 2: dense burst — mm, drain+t on two PSUM lanes, DMA out
            ob = obuf_pool.tile([D, r], BF16)
            for i, (c0, cw) in enumerate(blocks):
                h = min(HALF, cw)
                if i < 2:
                    ps_a = pre_a[i]
                else:
                    ps_a = mma_pool.tile([D, HALF], F32, tag="psA")
                    for s0 in range(0, h, MM_N):
                        sw = min(MM_N, h - s0)
                        nc.tensor.matmul(
                            ps_a[:, s0 : s0 + sw],
                            w1_sb[:],
                            x_sb[:, c0 + s0 : c0 + s0 + sw],
                        )
                if cw > h:
                    if i == 0:
                        ps_b = pre_b0
                    else:
                        ps_b = mmb_pool.tile([D, HALF], F32, tag="psB")
                        for s0 in range(h, cw, MM_N):
                            nc.tensor.matmul(
                                ps_b[:, s0 - h : s0 - h + MM_N],
                                w1_sb[:],
                                x_sb[:, c0 + s0 : c0 + s0 + MM_N],
                            )
                # DVE lane: ob = psA + t (1x from PSUM)
                nc.vector.tensor_scalar(
                    out=ob[:, c0 : c0 + h],
                    in0=ps_a[:, :h],
                    scalar1=t_sb[:],
                    scalar2=None,
                    op0=mybir.AluOpType.add,
                )
                if cw > h:
                    # ACT lane: ob = psB + t
                    nc.scalar.activation(
                        ob[:, c0 + h : c0 + cw],
                        ps_b[:, : cw - h],
                        mybir.ActivationFunctionType.Identity,
                        bias=t_sb[:],
                        scale=1.0,
                    )
                (nc.sync if i % 2 == 0 else nc.scalar).dma_start(
                    out[:, c0 : c0 + cw], ob[:, c0 : c0 + cw]
                )

    # Protocol signal + waits, invisible to Tile's scheduling sim:
    #  - cs_sem inc right after the colsum reduce on DVE
    #  - trigger must not fire before the local colsum is written
    #  - the gather reduce must not read before all 7 remote slots landed
    inc_cs = nc.vector.sem_inc(cs_sem, 1)
    _move_after(nc, inc_cs.ins, cs_reduce.ins)
    w_cs = nc.gpsimd.wait_ge(cs_sem, 1)
    _move_before(nc, w_cs.ins, trig.ins)
    w_arr = nc.vector.wait_ge(gsem, 14)
    _move_before(nc, w_arr.ins, gcs_reduce.ins)
    w_warm = nc.tensor.wait_ge(gsem, 2)
    _move_before(nc, w_warm.ins, first_dummy.ins)

    nc.compile()
    return nc


_nc_cache: dict = {}


def _get_nc(r: int):
    if r not in _nc_cache:
        _nc_cache[r] = build_nc(r)
    return _nc_cache[r]


LAST_RESULTS = None


def _execute(x, w1, w2, bias, r, trace=False, tmpdir=None, trace_cores=None):
    global LAST_RESULTS
    x = np.ascontiguousarray(np.asarray(x, dtype=np.float32))
    w1 = np.ascontiguousarray(np.asarray(w1, dtype=np.float32))
    w2 = np.ascontiguousarray(np.asarray(w2, dtype=np.float32))
    bias = np.asarray(bias, dtype=np.float32)
    n, d = x.shape
    assert d == D and r * N_CORES >= n

    xp = np.zeros((N_CORES * r, d), dtype=np.float32)
    xp[:n] = x
    # (8, r, d) -> (8, d, r) pre-transposed bf16 shards
    xts = np.ascontiguousarray(
        xp.reshape(N_CORES, r, d).transpose(0, 2, 1)
    ).astype(NP_BF16)
    wpack = np.ascontiguousarray(
        np.concatenate(
            [
                w1,
                w2 / float(n),
                bias.reshape(1, d).T,
                np.zeros((d, 1), np.float32),
            ],
            axis=1,
        )
    ).astype(NP_BF16)

    in_maps = [{"xt": xts[i], "wpack": wpack} for i in range(N_CORES)]

    nc = _get_nc(r)
    kwargs = {}
    if trace:
        kwargs.update(trace=True, tmpdir=tmpdir)
        if trace_cores is not None:
            kwargs.update(trace_cores=trace_cores)

    # A wedged device / flaky launch occasionally corrupts one core's run
    # (non-finite or wildly out-of-range outputs) or raises outright.
    # Detect and re-execute: y values are O(100), so 1e4 is a generous
    # sanity bound.
    y = None
    for attempt in range(3):
        try:
            res = run_bass_kernel_spmd(
                nc, in_maps, core_ids=list(range(N_CORES)), **kwargs
            )
        except Exception:
            if attempt == 2:
                raise
            continue
        LAST_RESULTS = res
        yts = [res.results[i]["out"] for i in range(N_CORES)]  # (D, r) bf16
        y = np.concatenate([yt.T for yt in yts], axis=0)[:n].astype(np.float32)
        if np.isfinite(y).all() and np.abs(y).max() < 1e4:
            break
    assert y is not None
    return np.ascontiguousarray(y)


def kernel(x, w1, w2, bias):
    return _execute(x, w1, w2, bias, R)
